# revision 32
# baseline (speedup 1.0000x reference)
"""Multi-head attention (B=2, S=4096, D=512, H=8) on 8 trn2 NeuronCores.

Sharding: head-pair tensor parallel. Core c handles batch c//4 and head
pair c%4 (two 64-dim heads = one 128-dim slice of D). Each core:
  - projects K^T [128, S] and V [S, 128] for its head pair only (no
    replication), Q^T per 512-wide query chunk,
  - runs causal flash-style attention over contiguous q-chunks in
    transposed layout: S^T[k,q] = K^T_slice.T @ Q^T_slice on PE, exp on
    ACT over two PSUM banks at a time, diagonal-band tiles multiplied by
    one of 4 constant SBUF masks on DVE, P^T @ V_aug on PE where V_aug
    carries a ones column so the softmax denominator falls out of the
    same matmul,
  - computes the output-projection partial for its 128 dims (+ bo/4),
  - ReduceScatters partials across the 4 cores of its batch per chunk.
X^T is laid out host-side; projection/attention matmuls for the next
chunk are interleaved into the current chunk's attention stream so the
PE stays continuously busy (p-state ramp). Matmuls run as float32r /
bf16 with fp32 PSUM accumulation.
"""

import numpy as np

# Problem dims (hardcoded per contract)
B, S, D, H, PD = 2, 4096, 512, 8, 64
P = 128
NCORES = 8
CPB = 4            # cores per batch (= head pairs)
HD = 128           # head-pair dim slice
DC = D // P        # 4 d-chunks of 128
QC = 512           # query-chunk width
NQC = S // QC      # 8
SC = 512           # seq chunk for K/V projection
NSC = S // SC      # 8
NKT = S // P       # 32 key tiles of 128

_prog_cache = {}


def _build(mode: str):
    import concourse.mybir as mybir
    import concourse.tile as tile
    from concourse import bacc

    f32 = mybir.dt.float32
    f32r = mybir.dt.float32r
    bf16 = mybir.dt.bfloat16
    Exp = mybir.ActivationFunctionType.Exp
    Alu = mybir.AluOpType

    nc = bacc.Bacc(debug=False, target_bir_lowering=False, num_devices=NCORES)

    xbT_d = nc.declare_dram_parameter("xbT", [P, DC, S], f32r, isOutput=False)
    wq_d = nc.declare_dram_parameter("wq", [P, DC, HD], f32r, isOutput=False)
    wk_d = nc.declare_dram_parameter("wk", [P, DC, HD], f32r, isOutput=False)
    wv_d = nc.declare_dram_parameter("wv", [P, DC, HD], f32r, isOutput=False)
    wo_d = nc.declare_dram_parameter("wo", [P, D], f32r, isOutput=False)
    bq_d = nc.declare_dram_parameter("bq", [P, 1], f32, isOutput=False)
    bk_d = nc.declare_dram_parameter("bk", [P, 1], f32, isOutput=False)
    bv_d = nc.declare_dram_parameter("bv", [P, HD], f32, isOutput=False)
    boq_d = nc.declare_dram_parameter("boq", [P, D], f32, isOutput=False)
    onesc_d = nc.declare_dram_parameter("onesc", [1, PD], f32r, isOutput=False)
    if mode == "tril":
        bm_d = nc.declare_dram_parameter("bandmask", [P, 4, QC], bf16,
                                         isOutput=False)
    elif mode == "add":
        maskT_d = nc.declare_dram_parameter("maskT", [S, S], f32,
                                            isOutput=False)
    out_d = nc.declare_dram_parameter("out", [NQC, P, D], bf16, isOutput=True)

    groups = [[0, 1, 2, 3], [4, 5, 6, 7]]

    with tile.TileContext(nc) as tc, nc.allow_low_precision(
            reason="float32r tiles are 4-byte fp32; PE rounds reads only"):
        with (
            tc.tile_pool(name="const", bufs=1) as constp,
            tc.tile_pool(name="pt", bufs=6) as ptp,
            tc.tile_pool(name="at", bufs=2) as atp,
            tc.tile_pool(name="osb", bufs=2) as osbp,
            tc.tile_pool(name="bcs", bufs=2) as bcsp,
            tc.tile_pool(name="rcp", bufs=2) as rcpp,
            tc.tile_pool(name="qk", bufs=2, space="PSUM") as qkps,
            tc.tile_pool(name="pj", bufs=2, space="PSUM") as pjps,
            tc.tile_pool(name="pv", bufs=2, space="PSUM") as pvps,
            tc.tile_pool(name="dram", bufs=1, space="DRAM") as dramp,
        ):
            # ---- constants / inputs ----
            xbt = constp.tile([P, DC, S], f32r, tag="xbt")
            wq = constp.tile([P, DC, HD], f32r, tag="wq")
            wk = constp.tile([P, DC, HD], f32r, tag="wk")
            wv = constp.tile([P, DC, HD], f32r, tag="wv")
            wo = constp.tile([P, D], f32r, tag="wo")
            bq = constp.tile([P, 1], f32, tag="bq")
            bk = constp.tile([P, 1], f32, tag="bk")
            bv = constp.tile([P, HD], f32, tag="bv")
            boq = constp.tile([P, D], f32, tag="boq")
            onesc = constp.tile([1, PD], f32r, tag="onesc")
            # wk/wv/bk/bv + the first xbT chunk head the queue so the first
            # K-projection can start as soon as possible
            for sb_t, dr_t in [(wk, wk_d), (wv, wv_d), (bk, bk_d),
                               (bv, bv_d)]:
                nc.sync.dma_start(sb_t[:], dr_t[:])
            nc.sync.dma_start(xbt[:, :, 0:SC], xbT_d[:, :, 0:SC])
            for sb_t, dr_t in [(wq, wq_d), (wo, wo_d), (bq, bq_d),
                               (boq, boq_d), (onesc, onesc_d)]:
                nc.sync.dma_start(sb_t[:], dr_t[:])
            if mode == "tril":
                bm = constp.tile([P, 4, QC], bf16, tag="bm")
                nc.sync.dma_start(bm[:], bm_d[:])

            # K^T for the head pair: kt[p, s] = K^T[hp*128+p, s]
            kt = constp.tile([P, S], bf16, tag="kt")
            # V tiles: vts[p, kti, h, 0:64] = V[kti*128+p, (2hp+h)*64 ...],
            # col 64 = 1.0 (softmax-denominator row of the PV matmul)
            vts = constp.tile([P, NKT, 2, PD + 1], bf16, tag="vts")
            nc.vector.memset(vts[:, :, :, PD:PD + 1], 1.0)
            # Q^T: qt[p, qc, j] = Q^T[hp*128+p, qc*QC+j]
            qt = constp.tile([P, NQC, QC], bf16, tag="qt")

            # out-proj partial bounces (collective in), rank-major rows.
            # Chunks are grouped into three ReduceScatter batches so the CC
            # channel runs few large calls, the last one tiny.
            RSB = [[7, 6, 5, 4], [3, 2, 1], [0]]
            rs_pos = {qc: (bi, i) for bi, b in enumerate(RSB)
                      for i, qc in enumerate(b)}
            rs_ins = [dramp.tile([CPB, len(b), P, D], bf16, tag=f"rsi{bi}",
                                 name=f"rsi{bi}")
                      for bi, b in enumerate(RSB)]
            rs_outs = [dramp.tile([len(b), P, D], bf16, tag=f"rso{bi}",
                                  name=f"rso{bi}")
                      for bi, b in enumerate(RSB)]
            # tiny warm-up collective: absorbs cross-core arrival skew while
            # the CC channel is otherwise idle
            warm_in = dramp.tile([1, 32], f32r, tag="warm_in")
            warm_out = dramp.tile([1, 8], f32r, tag="warm_out")
            nc.sync.dma_start(warm_in[:], onesc_d[0:1, 0:32])

            for sc in range(1, NSC):
                nc.sync.dma_start(xbt[:, :, sc * SC:(sc + 1) * SC],
                                  xbT_d[:, :, sc * SC:(sc + 1) * SC])

            # ---- emission units (deferred so they can be interleaved) ----
            def kproj(sc):
                psk = pjps.tile([P, QC], f32, tag="pj")
                for dc in range(DC):
                    nc.tensor.matmul(
                        psk[:], wk[:, dc, :], xbt[:, dc, sc * SC:(sc + 1) * SC],
                        start=(dc == 0), stop=(dc == DC - 1))
                nc.vector.tensor_scalar_add(
                    kt[:, sc * SC:(sc + 1) * SC], psk[:], bk[:])

            def vproj(sc, st):
                psv = pjps.tile([P, QC], f32, tag="pj")
                s0 = sc * SC + st * P
                for dc in range(DC):
                    nc.tensor.matmul(
                        psv[:, 0:HD], xbt[:, dc, s0:s0 + P], wv[:, dc, :],
                        start=(dc == 0), stop=(dc == DC - 1))
                nc.vector.tensor_add(
                    out=vts[:, sc * (SC // P) + st, :, 0:PD],
                    in0=psv[:, 0:HD].rearrange("p (h d) -> p h d", h=2),
                    in1=bv[:].rearrange("p (h d) -> p h d", h=2))

            def qproj_unit(sc):
                def f():
                    psq = pjps.tile([P, QC], f32, tag="pj")
                    for dc in range(DC):
                        nc.tensor.matmul(
                            psq[:], wq[:, dc, :], xbt[:, dc, sc * QC:(sc + 1) * QC],
                            start=(dc == 0), stop=(dc == DC - 1))
                    nc.vector.tensor_scalar_add(qt[:, sc, :], psq[:], bq[:])
                return f

            def outproj_units(qc, at_tile):
                """Output projection for finished chunk qc (+ the batch's
                ReduceScatter when qc closes its batch)."""
                bi, pos = rs_pos[qc]

                def rt_unit(rt):
                    def f():
                        psf = pjps.tile([P, D], f32, tag="pj")
                        nc.tensor.matmul(
                            psf[:], at_tile[:, rt * P:(rt + 1) * P], wo[:],
                            start=True, stop=True)
                        osb = osbp.tile([P, D], bf16, tag="osb")
                        nc.vector.tensor_add(out=osb[:], in0=psf[:], in1=boq[:])
                        nc.sync.dma_start(rs_ins[bi][rt, pos], osb[:])
                    return f

                def rs_unit():
                    nc.gpsimd.collective_compute(
                        "ReduceScatter", Alu.add, replica_groups=groups,
                        ins=[rs_ins[bi].opt()], outs=[rs_outs[bi].opt()])
                    # gpsimd queue: stays ordered after the RS without
                    # blocking the sync queue's data-path DMAs
                    for i, q in enumerate(RSB[bi]):
                        nc.gpsimd.dma_start(out_d[q], rs_outs[bi][i])

                units = [rt_unit(0), rt_unit(1), rt_unit(2), rt_unit(3)]
                if pos == len(RSB[bi]) - 1:
                    units.append(rs_unit)
                return units

            # ---- prefix: project K (all) and V (all but last chunk), Q for
            # chunk 7; PE ramps to full p-state on this dense matmul run ----
            for sc in range(NSC):
                kproj(sc)
                if sc == 0:
                    nc.gpsimd.collective_compute(
                        "ReduceScatter", Alu.add, replica_groups=groups,
                        ins=[warm_in.opt()], outs=[warm_out.opt()])
                if sc < NSC - 1:
                    for st in range(4):
                        vproj(sc, st)
            qproj_unit(NQC - 1)()
            # V for the last seq chunk becomes chunk-7 filler work (only
            # needed by the final PV pairs)
            pending = [(lambda st: (lambda: vproj(NSC - 1, st)))(st)
                       for st in range(4)]

            # ---- main loop: attention chunks largest-first so every
            # ReduceScatter except the last tiny one overlaps compute ----
            for qc in range(NQC - 1, -1, -1):
                if qc > 0:
                    pending.append(qproj_unit(qc - 1))
                nkt_c = 4 * qc + 4 if mode == "tril" else NKT
                npairs_h = nkt_c // 2
                npairs = 2 * npairs_h
                at_tile = atp.tile([P, QC], f32r, tag="at")
                pair_idx = 0
                fill_emitted = 0
                for h in range(2):
                    pvt = pvps.tile([PD + 1, QC], f32, tag="pv",
                                    name=f"pv{qc}_{h}")
                    # lag-1 software pipeline: PV of pair i-1 is emitted
                    # after QK of pair i, so PV never waits on its own EXP
                    prev_pt2 = None
                    for i in range(npairs_h):
                        qkt = qkps.tile([P, 2, QC], f32, tag="qk")
                        for j in range(2):
                            kc = 2 * i + j
                            nc.tensor.matmul(
                                qkt[:, j, :],
                                kt[h * PD:(h + 1) * PD, kc * P:(kc + 1) * P],
                                qt[h * PD:(h + 1) * PD, qc, :],
                                start=True, stop=True)
                        if mode == "add":
                            mt = ptp.tile([P, 2, QC], f32, tag="mt")
                            nc.sync.dma_start(
                                mt[:],
                                maskT_d[2 * i * P:(2 * i + 2) * P,
                                        qc * QC:(qc + 1) * QC].rearrange(
                                            "(t p) q -> p t q", p=P))
                            nc.vector.scalar_tensor_tensor(
                                out=qkt[:], in0=mt[:], scalar=-1e9,
                                in1=qkt[:], op0=Alu.mult, op1=Alu.add)
                        banded = mode == "tril" and 2 * i >= 4 * qc
                        pt2 = ptp.tile([P, 2, QC], bf16, tag="pt")
                        if banded:
                            pr2 = ptp.tile([P, 2, QC], bf16, tag="pt")
                            nc.scalar.activation(pr2[:], qkt[:], Exp,
                                                 scale=0.125)
                            t0 = 2 * i - 4 * qc
                            nc.vector.tensor_mul(
                                out=pt2[:], in0=pr2[:],
                                in1=bm[:, t0:t0 + 2, :])
                        else:
                            nc.scalar.activation(pt2[:], qkt[:], Exp,
                                                 scale=0.125)
                        if prev_pt2 is not None:
                            for j in range(2):
                                kc = 2 * (i - 1) + j
                                nc.tensor.matmul(
                                    pvt[:], vts[:, kc, h, :], prev_pt2[:, j, :],
                                    start=(kc == 0), stop=False,
                                    skip_group_check=True)
                        prev_pt2 = pt2
                        # interleave deferred projection/output work so the
                        # PE never idles while ACT catches up
                        pair_idx += 1
                        want = len(pending) * pair_idx // npairs
                        while fill_emitted < want:
                            pending[fill_emitted]()
                            fill_emitted += 1
                    for j in range(2):
                        kc = 2 * (npairs_h - 1) + j
                        nc.tensor.matmul(
                            pvt[:], vts[:, kc, h, :], prev_pt2[:, j, :],
                            start=(kc == 0), stop=(kc == nkt_c - 1),
                            skip_group_check=True)
                    # normalize head h: rows 0:64 / row 64
                    dn = rcpp.tile([1, QC], f32r, tag="rcp")
                    nc.vector.tensor_copy(out=dn[:], in_=pvt[PD:PD + 1, :])
                    bcp = pjps.tile([P, QC], f32, tag="pj")
                    nc.tensor.matmul(bcp[0:PD, :], onesc[:], dn[:],
                                     start=True, stop=True)
                    bcs = bcsp.tile([PD, QC], f32, tag="bcs")
                    nc.vector.reciprocal_approx_fast(
                        out=bcs[:], in_=bcp[0:PD, :])
                    nc.vector.tensor_mul(
                        out=at_tile[h * PD:(h + 1) * PD, :],
                        in0=pvt[0:PD, :], in1=bcs[:])
                for u in pending[fill_emitted:]:
                    u()
                pending = outproj_units(qc, at_tile)
            for u in pending:
                u()
    nc.finalize()
    return nc


def _get_prog(mode: str):
    if mode not in _prog_cache:
        _prog_cache[mode] = _build(mode)
    return _prog_cache[mode]


def make_in_maps(inputs, mask, Wq, bq, Wk, bk, Wv, bv, Wo, bo):
    import ml_dtypes
    inputs = np.asarray(inputs, dtype=np.float32)
    mask = np.asarray(mask, dtype=np.float32)
    if not np.any(mask):
        mode = "none"
    elif np.array_equal(mask, np.triu(np.ones((S, S), dtype=np.float32), 1)):
        mode = "tril"
    else:
        mode = "add"

    Wq, Wk, Wv, Wo = (np.asarray(a, dtype=np.float32) for a in (Wq, Wk, Wv, Wo))
    bqv, bkv, bvv, bov = (np.asarray(a, dtype=np.float32) for a in (bq, bk, bv, bo))

    if mode == "tril":
        pp, tt, qq = np.ogrid[0:P, 0:4, 0:QC]
        bandmask = (qq >= tt * P + pp).astype(ml_dtypes.bfloat16)
    elif mode == "add":
        maskT = np.ascontiguousarray(mask.T)

    xbTs = []
    for b in range(B):
        xbTs.append(np.ascontiguousarray(
            inputs[b].T.reshape(DC, P, S).transpose(1, 0, 2)))

    def wslice(W, hp):
        return np.ascontiguousarray(
            W[:, hp * HD:(hp + 1) * HD].reshape(DC, P, HD).transpose(1, 0, 2))

    in_maps = []
    for c in range(NCORES):
        b, hp = c // CPB, c % CPB
        m = {
            "xbT": xbTs[b],
            "wq": wslice(Wq, hp), "wk": wslice(Wk, hp), "wv": wslice(Wv, hp),
            "wo": np.ascontiguousarray(Wo[hp * HD:(hp + 1) * HD, :]),
            "bq": np.ascontiguousarray(bqv[hp * HD:(hp + 1) * HD, None]),
            "bk": np.ascontiguousarray(bkv[hp * HD:(hp + 1) * HD, None]),
            "bv": np.ascontiguousarray(
                np.broadcast_to(bvv[hp * HD:(hp + 1) * HD], (P, HD))),
            "boq": np.ascontiguousarray(
                np.broadcast_to(bov / CPB, (P, D))),
            "onesv": np.ones((P, NKT, 2, 1), dtype=ml_dtypes.bfloat16),
            "onesc": np.ones((1, PD), dtype=np.float32),
        }
        if mode == "tril":
            m["bandmask"] = bandmask
        elif mode == "add":
            m["maskT"] = maskT
        in_maps.append(m)
    return mode, in_maps


def assemble(results, mode):
    out = np.empty((B, S, D), dtype=np.float32)
    for c in range(NCORES):
        b, r = c // CPB, c % CPB
        rows = (np.arange(NQC)[:, None] * QC + r * P
                + np.arange(P)[None, :]).ravel()
        out[b, rows] = results[c]["out"].reshape(NQC * P, D).astype(np.float32)
    return out


def kernel(inputs, mask, Wq, bq, Wk, bk, Wv, bv, Wo, bo):
    from concourse.bass_utils import run_bass_kernel_spmd

    mode, in_maps = make_in_maps(inputs, mask, Wq, bq, Wk, bk, Wv, bv, Wo, bo)
    nc = _get_prog(mode)
    res = run_bass_kernel_spmd(nc, in_maps, core_ids=list(range(NCORES)))
    return assemble(res.results, mode)


# revision 37
# speedup vs baseline: 1.0310x; 1.0310x over previous
"""Multi-head attention (B=2, S=4096, D=512, H=8) on 8 trn2 NeuronCores.

Sharding: head-pair tensor parallel. Core c handles batch c//4 and head
pair c%4 (two 64-dim heads = one 128-dim slice of D). Each core:
  - projects K^T [128, S] and V [S, 128] for its head pair only (no
    replication), Q^T per 512-wide query chunk,
  - runs causal flash-style attention over contiguous q-chunks in
    transposed layout: S^T[k,q] = K^T_slice.T @ Q^T_slice on PE, exp on
    ACT over two PSUM banks at a time, diagonal-band tiles multiplied by
    one of 4 constant SBUF masks on DVE, P^T @ V_aug on PE where V_aug
    carries a ones column so the softmax denominator falls out of the
    same matmul,
  - computes the output-projection partial for its 128 dims (+ bo/4),
  - ReduceScatters partials across the 4 cores of its batch per chunk.
X^T is laid out host-side; projection/attention matmuls for the next
chunk are interleaved into the current chunk's attention stream so the
PE stays continuously busy (p-state ramp). Matmuls run as float32r /
bf16 with fp32 PSUM accumulation.
"""

import numpy as np

# Problem dims (hardcoded per contract)
B, S, D, H, PD = 2, 4096, 512, 8, 64
P = 128
NCORES = 8
CPB = 4            # cores per batch (= head pairs)
HD = 128           # head-pair dim slice
DC = D // P        # 4 d-chunks of 128
QC = 512           # query-chunk width
NQC = S // QC      # 8
SC = 512           # seq chunk for K/V projection
NSC = S // SC      # 8
NKT = S // P       # 32 key tiles of 128

_prog_cache = {}


def _build(mode: str):
    import concourse.mybir as mybir
    import concourse.tile as tile
    from concourse import bacc

    f32 = mybir.dt.float32
    f32r = mybir.dt.float32r
    bf16 = mybir.dt.bfloat16
    Exp = mybir.ActivationFunctionType.Exp
    Alu = mybir.AluOpType

    nc = bacc.Bacc(debug=False, target_bir_lowering=False, num_devices=NCORES)

    xbT_d = nc.declare_dram_parameter("xbT", [P, DC, S], f32r, isOutput=False)
    wq_d = nc.declare_dram_parameter("wq", [P, DC, HD], f32r, isOutput=False)
    wk_d = nc.declare_dram_parameter("wk", [P, DC, HD], f32r, isOutput=False)
    wv_d = nc.declare_dram_parameter("wv", [P, DC, HD], f32r, isOutput=False)
    wo_d = nc.declare_dram_parameter("wo", [P, D], f32r, isOutput=False)
    bq_d = nc.declare_dram_parameter("bq", [P, 1], f32, isOutput=False)
    bk_d = nc.declare_dram_parameter("bk", [P, 1], f32, isOutput=False)
    bv_d = nc.declare_dram_parameter("bv", [P, HD], f32, isOutput=False)
    boq_d = nc.declare_dram_parameter("boq", [P, D], f32, isOutput=False)
    onesc_d = nc.declare_dram_parameter("onesc", [1, PD], f32r, isOutput=False)
    if mode == "tril":
        bm_d = nc.declare_dram_parameter("bandmask", [P, 4, QC], bf16,
                                         isOutput=False)
    elif mode == "add":
        maskT_d = nc.declare_dram_parameter("maskT", [S, S], f32,
                                            isOutput=False)
    out_d = nc.declare_dram_parameter("out", [NQC, P, D], bf16, isOutput=True)

    groups = [[0, 1, 2, 3], [4, 5, 6, 7]]

    with tile.TileContext(nc) as tc, nc.allow_low_precision(
            reason="float32r tiles are 4-byte fp32; PE rounds reads only"):
        with (
            tc.tile_pool(name="const", bufs=1) as constp,
            tc.tile_pool(name="pt", bufs=6) as ptp,
            tc.tile_pool(name="at", bufs=2) as atp,
            tc.tile_pool(name="osb", bufs=2) as osbp,
            tc.tile_pool(name="bcs", bufs=2) as bcsp,
            tc.tile_pool(name="rcp", bufs=2) as rcpp,
            tc.tile_pool(name="qk", bufs=2, space="PSUM") as qkps,
            tc.tile_pool(name="pj", bufs=2, space="PSUM") as pjps,
            tc.tile_pool(name="pv", bufs=2, space="PSUM") as pvps,
            tc.tile_pool(name="dram", bufs=1, space="DRAM") as dramp,
        ):
            # ---- constants / inputs ----
            xbt = constp.tile([P, DC, S], f32r, tag="xbt")
            wq = constp.tile([P, DC, HD], f32r, tag="wq")
            wk = constp.tile([P, DC, HD], f32r, tag="wk")
            wv = constp.tile([P, DC, HD], f32r, tag="wv")
            wo = constp.tile([P, D], f32r, tag="wo")
            bq = constp.tile([P, 1], f32, tag="bq")
            bk = constp.tile([P, 1], f32, tag="bk")
            bv = constp.tile([P, HD], f32, tag="bv")
            boq = constp.tile([P, D], f32, tag="boq")
            onesc = constp.tile([1, PD], f32r, tag="onesc")
            # wk/wv/bk/bv + the first xbT chunk head the queue so the first
            # K-projection can start as soon as possible
            for sb_t, dr_t in [(wk, wk_d), (wv, wv_d), (bk, bk_d),
                               (bv, bv_d)]:
                nc.sync.dma_start(sb_t[:], dr_t[:])
            nc.sync.dma_start(xbt[:, :, 0:SC], xbT_d[:, :, 0:SC])
            for sb_t, dr_t in [(wq, wq_d), (wo, wo_d), (bq, bq_d),
                               (boq, boq_d), (onesc, onesc_d)]:
                nc.sync.dma_start(sb_t[:], dr_t[:])
            if mode == "tril":
                bm = constp.tile([P, 4, QC], bf16, tag="bm")
                nc.sync.dma_start(bm[:], bm_d[:])

            # K^T for the head pair: kt[p, s] = K^T[hp*128+p, s]
            kt = constp.tile([P, S], bf16, tag="kt")
            # V tiles: vts[p, kti, h, 0:64] = V[kti*128+p, (2hp+h)*64 ...],
            # col 64 = 1.0 (softmax-denominator row of the PV matmul)
            vts = constp.tile([P, NKT, 2, PD + 1], bf16, tag="vts")
            nc.vector.memset(vts[:, :, :, PD:PD + 1], 1.0)
            # Q^T: qt[p, qc, j] = Q^T[hp*128+p, qc*QC+j]
            qt = constp.tile([P, NQC, QC], bf16, tag="qt")

            # out-proj partial bounce (collective in), rank-major rows
            rs_in = dramp.tile([NQC, CPB, P, D], bf16, tag="rs_in")
            rs_out = dramp.tile([NQC, P, D], bf16, tag="rs_out")


            for sc in range(1, NSC):
                nc.sync.dma_start(xbt[:, :, sc * SC:(sc + 1) * SC],
                                  xbT_d[:, :, sc * SC:(sc + 1) * SC])

            # ---- emission units (deferred so they can be interleaved) ----
            def kproj(sc):
                psk = pjps.tile([P, QC], f32, tag="pj")
                for dc in range(DC):
                    nc.tensor.matmul(
                        psk[:], wk[:, dc, :], xbt[:, dc, sc * SC:(sc + 1) * SC],
                        start=(dc == 0), stop=(dc == DC - 1))
                nc.vector.tensor_scalar_add(
                    kt[:, sc * SC:(sc + 1) * SC], psk[:], bk[:])

            def vproj(sc, st):
                psv = pjps.tile([P, QC], f32, tag="pj")
                s0 = sc * SC + st * P
                for dc in range(DC):
                    nc.tensor.matmul(
                        psv[:, 0:HD], xbt[:, dc, s0:s0 + P], wv[:, dc, :],
                        start=(dc == 0), stop=(dc == DC - 1))
                nc.vector.tensor_add(
                    out=vts[:, sc * (SC // P) + st, :, 0:PD],
                    in0=psv[:, 0:HD].rearrange("p (h d) -> p h d", h=2),
                    in1=bv[:].rearrange("p (h d) -> p h d", h=2))

            def qproj_unit(sc):
                def f():
                    psq = pjps.tile([P, QC], f32, tag="pj")
                    for dc in range(DC):
                        nc.tensor.matmul(
                            psq[:], wq[:, dc, :], xbt[:, dc, sc * QC:(sc + 1) * QC],
                            start=(dc == 0), stop=(dc == DC - 1))
                    nc.vector.tensor_scalar_add(qt[:, sc, :], psq[:], bq[:])
                return f

            def outproj_units(qc, at_tile):
                """Output projection + ReduceScatter for finished chunk qc."""
                def rt_unit(rt):
                    def f():
                        psf = pjps.tile([P, D], f32, tag="pj")
                        nc.tensor.matmul(
                            psf[:], at_tile[:, rt * P:(rt + 1) * P], wo[:],
                            start=True, stop=True)
                        osb = osbp.tile([P, D], bf16, tag="osb")
                        nc.vector.tensor_add(out=osb[:], in0=psf[:], in1=boq[:])
                        nc.sync.dma_start(rs_in[qc, rt], osb[:])
                    return f

                def rs_unit():
                    nc.gpsimd.collective_compute(
                        "ReduceScatter", Alu.add, replica_groups=groups,
                        ins=[rs_in[qc].opt()], outs=[rs_out[qc].opt()])
                    # gpsimd queue: stays ordered after the RS without
                    # blocking the sync queue's data-path DMAs
                    nc.gpsimd.dma_start(out_d[qc], rs_out[qc])

                return [rt_unit(0), rt_unit(1), rt_unit(2), rt_unit(3), rs_unit]

            # ---- prefix: project K (all) and V (all but last chunk), Q for
            # chunk 7; PE ramps to full p-state on this dense matmul run ----
            for sc in range(NSC):
                kproj(sc)
                if sc < NSC - 1:
                    for st in range(4):
                        vproj(sc, st)
            qproj_unit(NQC - 1)()
            # V for the last seq chunk becomes chunk-7 filler work (only
            # needed by the final PV pairs)
            pending = [(lambda st: (lambda: vproj(NSC - 1, st)))(st)
                       for st in range(4)]

            # ---- main loop: big and small attention chunks interleaved so
            # per-chunk ReduceScatter traffic stays spread out and every RS
            # except the final one hides under compute ----
            ORDER = [7, 0, 6, 1, 5, 2, 4, 3]
            for ci, qc in enumerate(ORDER):
                if ci + 1 < NQC:
                    pending.append(qproj_unit(ORDER[ci + 1]))
                nkt_c = 4 * qc + 4 if mode == "tril" else NKT
                npairs_h = nkt_c // 2
                npairs = 2 * npairs_h
                at_tile = atp.tile([P, QC], f32r, tag="at")
                pair_idx = 0
                fill_emitted = 0
                for h in range(2):
                    pvt = pvps.tile([PD + 1, QC], f32, tag="pv",
                                    name=f"pv{qc}_{h}")
                    # lag-1 software pipeline: PV of pair i-1 is emitted
                    # after QK of pair i, so PV never waits on its own EXP
                    prev_pt2 = None
                    for i in range(npairs_h):
                        qkt = qkps.tile([P, 2, QC], f32, tag="qk")
                        for j in range(2):
                            kc = 2 * i + j
                            nc.tensor.matmul(
                                qkt[:, j, :],
                                kt[h * PD:(h + 1) * PD, kc * P:(kc + 1) * P],
                                qt[h * PD:(h + 1) * PD, qc, :],
                                start=True, stop=True)
                        if mode == "add":
                            mt = ptp.tile([P, 2, QC], f32, tag="mt")
                            nc.sync.dma_start(
                                mt[:],
                                maskT_d[2 * i * P:(2 * i + 2) * P,
                                        qc * QC:(qc + 1) * QC].rearrange(
                                            "(t p) q -> p t q", p=P))
                            nc.vector.scalar_tensor_tensor(
                                out=qkt[:], in0=mt[:], scalar=-1e9,
                                in1=qkt[:], op0=Alu.mult, op1=Alu.add)
                        banded = mode == "tril" and 2 * i >= 4 * qc
                        pt2 = ptp.tile([P, 2, QC], bf16, tag="pt")
                        if banded:
                            pr2 = ptp.tile([P, 2, QC], bf16, tag="pt")
                            nc.scalar.activation(pr2[:], qkt[:], Exp,
                                                 scale=0.125)
                            t0 = 2 * i - 4 * qc
                            nc.vector.tensor_mul(
                                out=pt2[:], in0=pr2[:],
                                in1=bm[:, t0:t0 + 2, :])
                        else:
                            nc.scalar.activation(pt2[:], qkt[:], Exp,
                                                 scale=0.125)
                        if prev_pt2 is not None:
                            for j in range(2):
                                kc = 2 * (i - 1) + j
                                nc.tensor.matmul(
                                    pvt[:], vts[:, kc, h, :], prev_pt2[:, j, :],
                                    start=(kc == 0), stop=False,
                                    skip_group_check=True)
                        prev_pt2 = pt2
                        # interleave deferred projection/output work so the
                        # PE never idles while ACT catches up
                        pair_idx += 1
                        want = len(pending) * pair_idx // npairs
                        while fill_emitted < want:
                            pending[fill_emitted]()
                            fill_emitted += 1
                    for j in range(2):
                        kc = 2 * (npairs_h - 1) + j
                        nc.tensor.matmul(
                            pvt[:], vts[:, kc, h, :], prev_pt2[:, j, :],
                            start=(kc == 0), stop=(kc == nkt_c - 1),
                            skip_group_check=True)
                    # normalize head h: rows 0:64 / row 64
                    dn = rcpp.tile([1, QC], f32r, tag="rcp")
                    nc.vector.tensor_copy(out=dn[:], in_=pvt[PD:PD + 1, :])
                    bcp = pjps.tile([P, QC], f32, tag="pj")
                    nc.tensor.matmul(bcp[0:PD, :], onesc[:], dn[:],
                                     start=True, stop=True)
                    bcs = bcsp.tile([PD, QC], f32, tag="bcs")
                    nc.vector.reciprocal_approx_fast(
                        out=bcs[:], in_=bcp[0:PD, :])
                    nc.vector.tensor_mul(
                        out=at_tile[h * PD:(h + 1) * PD, :],
                        in0=pvt[0:PD, :], in1=bcs[:])
                for u in pending[fill_emitted:]:
                    u()
                pending = outproj_units(qc, at_tile)
            for u in pending:
                u()
    nc.finalize()
    return nc


def _get_prog(mode: str):
    if mode not in _prog_cache:
        _prog_cache[mode] = _build(mode)
    return _prog_cache[mode]


def make_in_maps(inputs, mask, Wq, bq, Wk, bk, Wv, bv, Wo, bo):
    import ml_dtypes
    inputs = np.asarray(inputs, dtype=np.float32)
    mask = np.asarray(mask, dtype=np.float32)
    if not np.any(mask):
        mode = "none"
    elif np.array_equal(mask, np.triu(np.ones((S, S), dtype=np.float32), 1)):
        mode = "tril"
    else:
        mode = "add"

    Wq, Wk, Wv, Wo = (np.asarray(a, dtype=np.float32) for a in (Wq, Wk, Wv, Wo))
    bqv, bkv, bvv, bov = (np.asarray(a, dtype=np.float32) for a in (bq, bk, bv, bo))

    if mode == "tril":
        pp, tt, qq = np.ogrid[0:P, 0:4, 0:QC]
        bandmask = (qq >= tt * P + pp).astype(ml_dtypes.bfloat16)
    elif mode == "add":
        maskT = np.ascontiguousarray(mask.T)

    xbTs = []
    for b in range(B):
        xbTs.append(np.ascontiguousarray(
            inputs[b].T.reshape(DC, P, S).transpose(1, 0, 2)))

    def wslice(W, hp):
        return np.ascontiguousarray(
            W[:, hp * HD:(hp + 1) * HD].reshape(DC, P, HD).transpose(1, 0, 2))

    in_maps = []
    for c in range(NCORES):
        b, hp = c // CPB, c % CPB
        m = {
            "xbT": xbTs[b],
            "wq": wslice(Wq, hp), "wk": wslice(Wk, hp), "wv": wslice(Wv, hp),
            "wo": np.ascontiguousarray(Wo[hp * HD:(hp + 1) * HD, :]),
            "bq": np.ascontiguousarray(bqv[hp * HD:(hp + 1) * HD, None]),
            "bk": np.ascontiguousarray(bkv[hp * HD:(hp + 1) * HD, None]),
            "bv": np.ascontiguousarray(
                np.broadcast_to(bvv[hp * HD:(hp + 1) * HD], (P, HD))),
            "boq": np.ascontiguousarray(
                np.broadcast_to(bov / CPB, (P, D))),
            "onesv": np.ones((P, NKT, 2, 1), dtype=ml_dtypes.bfloat16),
            "onesc": np.ones((1, PD), dtype=np.float32),
        }
        if mode == "tril":
            m["bandmask"] = bandmask
        elif mode == "add":
            m["maskT"] = maskT
        in_maps.append(m)
    return mode, in_maps


def assemble(results, mode):
    out = np.empty((B, S, D), dtype=np.float32)
    for c in range(NCORES):
        b, r = c // CPB, c % CPB
        rows = (np.arange(NQC)[:, None] * QC + r * P
                + np.arange(P)[None, :]).ravel()
        out[b, rows] = results[c]["out"].reshape(NQC * P, D).astype(np.float32)
    return out


def kernel(inputs, mask, Wq, bq, Wk, bk, Wv, bv, Wo, bo):
    from concourse.bass_utils import run_bass_kernel_spmd

    mode, in_maps = make_in_maps(inputs, mask, Wq, bq, Wk, bk, Wv, bv, Wo, bo)
    nc = _get_prog(mode)
    res = run_bass_kernel_spmd(nc, in_maps, core_ids=list(range(NCORES)))
    return assemble(res.results, mode)
